# revision 19
# baseline (speedup 1.0000x reference)
"""Trainium2 Bass kernel for nn_BusStopPredictor (2-layer GCN + sigmoid head).

kernel(**inputs) takes FULL inputs, shards across 8 NeuronCores internally,
and returns the FULL [200000] output.

Strategy (graph/data parallel, dst-sharded, grid-ordered storage):
  - nodes assigned round-robin by in-degree rank (balances degree-class
    counts across cores); per-core node storage order = phase-1 grid order
    (nodes grouped by in-degree class, padded to 128-chunks)
  - phase 1 (2-wide x aggregation) uses NO indexed DMA: the per-edge
    x[src] values are materialized host-side in slot order (a static
    permutation of the input = halo materialization), so on device it is a
    dense load + DVE multiply by dinv[src] + fixed-stride segmented
    tensor_reduce producing xa directly in storage order
  - GCN algebra folded as in:
      xa    = sum dinv[s]*x[s]  (in-edges)
      xaug  = [dinv^2*(xa + dinv*x_self), dinv];  W1aug=[W1;b1]
      h1'   = relu(xaug@W1aug)  ( = dinv*h1 )
      g'    = h1'@W2            ( = dinv*(h1@W2) )  -> AllGather
      out2  = sum_in g'[s] + g'[self];  h2 = relu(dinv*out2 + b2)
      y     = sigmoid(h2@Wp + bp)
  - phase 2 (64-wide g' aggregation) via SWDGE dma_gather per src bucket
    (int16 idx, 256B rows) + degree-class tensor_reduce + dma_scatter_add
    (CCE add) into 2 ping-pong HBM accumulators; accumulator 0 is
    pre-loaded with the self term g' by the matmul output path
  - gathers on SWDGE queue 0, scatters on queue 1, with an enlarged
    descriptor ring so desc-gen is not throttled by transfer drain
"""

import numpy as np

N = 200000
NCORES = 8
NLOC = N // NCORES          # 25000
P = 128
DG = 64                     # g feature width
CALL_MAX = 1024             # max slots per SWDGE call (64 desc/lane packet cap)
SCRATCH = 49152             # SWDGE descriptor ring carveout (bytes/partition)


# ----------------------------------------------------------------- host prep

def _prep(edge_index):
    src = np.asarray(edge_index[0], np.int64)
    dst = np.asarray(edge_index[1], np.int64)
    indeg = np.bincount(dst, minlength=N)
    deg = indeg + 1
    dinv = (1.0 / np.sqrt(deg)).astype(np.float32)

    # --- node -> (core, row) assignment: round-robin by in-degree rank ---
    order = np.argsort(indeg, kind="stable")      # ascending indeg
    core_of = np.empty(N, np.int32)
    core_of[order] = np.arange(N, dtype=np.int32) % NCORES
    # per-core nodes in ascending-indeg order
    nodes_by_core = [order[core_of[order] == c] for c in range(NCORES)]

    # --- phase-1 grid: classes by total indeg, uniform chunk counts ---
    dmax1 = int(indeg.max())
    cnt1 = np.zeros((NCORES, dmax1 + 1), np.int64)
    for c in range(NCORES):
        cnt1[c] = np.bincount(indeg[nodes_by_core[c]], minlength=dmax1 + 1)
    chunks1 = np.zeros(dmax1 + 1, np.int64)
    for d in range(dmax1 + 1):
        m = int(cnt1[:, d].max())
        if m:
            chunks1[d] = (m + P - 1) // P

    grid_rows = int(chunks1.sum()) * P
    R_BLK = ((grid_rows + 511) // 512) * 512
    G1 = R_BLK // P
    TRASH = R_BLK

    # row assignment per core (all cores share class bases)
    class_base = np.zeros(dmax1 + 2, np.int64)
    np.cumsum(chunks1 * P, out=class_base[1:])
    row_of = np.full(N, -1, np.int64)     # local row within core block
    for c in range(NCORES):
        nb = nodes_by_core[c]
        degs = indeg[nb]                  # ascending
        pos = np.zeros(len(nb), np.int64)
        for d in range(dmax1 + 1):
            sel = degs == d
            k = int(sel.sum())
            if k:
                pos[sel] = class_base[d] + np.arange(k)
        row_of[nb] = pos

    # phase-1 slot layout: class-major, unit (d) has chunks1[d] chunks
    slot_base = {}
    so = 0
    units1 = []
    for d in range(1, dmax1 + 1):
        if chunks1[d]:
            slot_base[d] = so
            units1.append((d, int(chunks1[d]), so, int(class_base[d])))
            so += int(chunks1[d]) * d * P
    S1 = so
    assert S1 % P == 0

    # per-core x_slots / norm1 fill (vectorized per class)
    adj_dst_order = np.argsort(dst, kind="stable")
    es_by_dst = src[adj_dst_order]                  # srcs grouped by dst
    starts = np.zeros(N + 1, np.int64)
    np.cumsum(indeg, out=starts[1:])

    x_slots_all = []
    norm1_all = []

    def fill_phase1(c, x):
        xs = np.zeros((S1 // P, P, 2), np.float32)
        nm = np.zeros((S1 // P, P), np.float32)
        nb = nodes_by_core[c]
        degs = indeg[nb]
        for d, gcnt, sbase, rbase in units1:
            nodes = nb[degs == d]
            if len(nodes) == 0:
                continue
            # node i (i-th of this class) at chunk g=i//P, partition p=i%P
            # slots: columns sbase//P + g*d + k, partition p
            idxs = np.arange(len(nodes))
            g = idxs // P
            p = idxs % P
            # all srcs for these nodes: [len(nodes), d]
            s0 = starts[nodes]
            srcs = es_by_dst[(s0[:, None] + np.arange(d)[None, :])]
            col = sbase // P + g * d
            for k in range(d):
                xs[col + k, p] = x[srcs[:, k]]
                nm[col + k, p] = dinv[srcs[:, k]]
        return xs.transpose(1, 0, 2).copy(), nm.T.copy()

    # --- phase-2 grids: per src-bucket degree-class structure ---
    src_core = core_of[src]
    src_row = row_of[src]
    dst_core = core_of[dst]
    dst_row = row_of[dst]

    # counts[c][b] over local rows; adjacency per (c,b) sorted by dst row
    counts = np.zeros((NCORES, NCORES, R_BLK), np.int32)
    adj2 = [[None] * NCORES for _ in range(NCORES)]
    for c in range(NCORES):
        em = dst_core == c
        ed = dst_row[em]
        es = src_row[em]
        eb = src_core[em]
        for b in range(NCORES):
            bm = eb == b
            edb = ed[bm]
            esb = es[bm]
            counts[c, b] = np.bincount(edb, minlength=R_BLK)
            o = np.argsort(edb, kind="stable")
            st2 = np.zeros(R_BLK + 1, np.int64)
            np.cumsum(counts[c, b], out=st2[1:])
            adj2[c][b] = (st2, esb[o])

    calls = []
    idx_parts = [[] for _ in range(NCORES)]
    sc_parts = [[] for _ in range(NCORES)]
    slot_off = 0
    row_off = 0

    for b in range(NCORES):
        dmax = int(counts[:, b].max())
        nch = {}
        for d in range(1, dmax + 1):
            m = int(max((counts[c, b] == d).sum() for c in range(NCORES)))
            if m:
                nch[d] = (m + P - 1) // P
        nodes_by_class = []
        for c in range(NCORES):
            dloc = counts[c, b]
            nodes_by_class.append({d: np.where(dloc == d)[0] for d in nch})

        pend_units = []
        pend_slots = 0

        def flush():
            nonlocal pend_units, pend_slots, slot_off, row_off
            if not pend_units:
                return
            units = []
            for d, g in pend_units:
                if units and units[-1][0] == d:
                    units[-1][1] += 1
                else:
                    units.append([d, 1])
            rows = sum(g for _, g in units) * P
            calls.append({
                "bucket": b,
                "slots": pend_slots,
                "rows": rows,
                "units": [(d, g) for d, g in units],
                "slot_off": slot_off,
                "row_off": row_off,
            })
            slot_off += pend_slots
            row_off += rows
            pend_units = []
            pend_slots = 0

        for d in sorted(nch):
            for g in range(nch[d]):
                if pend_slots + d * P > CALL_MAX:
                    flush()
                pend_units.append((d, g))
                pend_slots += d * P
        flush()

        for c in range(NCORES):
            st2, es_s = adj2[c][b]
            for d in sorted(nch):
                nodes = nodes_by_class[c][d]
                padded = nch[d] * P
                nodes_p = np.full(padded, -1, np.int64)
                nodes_p[:len(nodes)] = nodes
                ss = np.zeros((nch[d], d, P), np.int16)
                scr = np.full((nch[d], P), TRASH, np.int16)
                if len(nodes):
                    idxs = np.arange(len(nodes))
                    g = idxs // P
                    p = idxs % P
                    s0 = st2[nodes]
                    srcs = es_s[(s0[:, None] + np.arange(d)[None, :])]
                    for k in range(d):
                        ss[g, k, p] = srcs[:, k]
                    scr[g, p] = nodes
                idx_parts[c].append(ss.reshape(-1))
                sc_parts[c].append(scr.reshape(-1))

    S2 = slot_off
    R2 = row_off

    percore = []
    for c in range(NCORES):
        percore.append({
            "idx": np.concatenate(idx_parts[c]),
            "sc": np.concatenate(sc_parts[c]),
        })

    meta = {
        "R_BLK": R_BLK, "G1": G1, "TRASH": TRASH,
        "S1": S1, "units1": units1, "S2": S2, "R2": R2,
        "calls": calls,
        "nodes_by_core": nodes_by_core, "row_of": row_of,
        "dinv": dinv, "fill_phase1": fill_phase1,
        "percore": percore,
    }
    return meta


def _wrap16(vals_i16):
    """[S] int16 -> [128, S/16] wrap layout (i%16 partition, i//16 free,
    replicated across the 8 core groups)."""
    v = np.asarray(vals_i16, np.int16)
    assert len(v) % 128 == 0
    w = v.reshape(len(v) // 16, 16).T
    return np.tile(w, (8, 1))


# ------------------------------------------------------------- device kernel

def _build_bass(meta):
    import concourse.bass as bass
    import concourse.mybir as mybir
    import concourse.tile as tile
    from concourse import bacc
    from concourse.masks import make_identity

    F32, I16 = mybir.dt.float32, mybir.dt.int16
    BF16 = mybir.dt.bfloat16
    AX = mybir.AxisListType
    OP = mybir.AluOpType
    ACTF = mybir.ActivationFunctionType

    R_BLK = meta["R_BLK"]
    G1 = meta["G1"]
    S1 = meta["S1"]
    S2 = meta["S2"]
    R2 = meta["R2"]
    units1 = meta["units1"]
    calls = meta["calls"]
    BUF_ROWS = R_BLK + P

    nc = bacc.Bacc(trn_type="TRN2", num_devices=NCORES,
                   dynamic_dma_scratch_size=32768,
                   num_swdge_queues=2)

    # inputs
    xslots_in = nc.dram_tensor("xslots", [P, S1 // P, 2], F32, kind="ExternalInput")
    norm1_in = nc.dram_tensor("norm1", [P, S1 // P], F32, kind="ExternalInput")
    x_self = nc.dram_tensor("x_self", [P, G1, 2], F32, kind="ExternalInput")
    dinv_pl = nc.dram_tensor("dinv_pl", [P, G1], F32, kind="ExternalInput")
    dinv_pl2 = nc.dram_tensor("dinv_pl2", [P, G1], F32, kind="ExternalInput")
    w1aug = nc.dram_tensor("w1aug", [3, 128], F32, kind="ExternalInput")
    w2 = nc.dram_tensor("w2", [128, DG], F32, kind="ExternalInput")
    wp_rep = nc.dram_tensor("wp_rep", [P, DG], F32, kind="ExternalInput")
    b2_rep = nc.dram_tensor("b2_rep", [P, DG], F32, kind="ExternalInput")
    bp_rep = nc.dram_tensor("bp_rep", [P, 1], F32, kind="ExternalInput")
    idx_in = nc.dram_tensor("idx", [P, S2 // 16], I16, kind="ExternalInput")
    sc_in = nc.dram_tensor("sc", [P, R2 // 16], I16, kind="ExternalInput")
    y_out = nc.dram_tensor("y", [R_BLK], F32, kind="ExternalOutput")

    with tile.TileContext(nc) as tc:
        with (
            tc.tile_pool(name="dram", bufs=1, space="DRAM") as dram,
            tc.tile_pool(name="const", bufs=1) as cp,
            tc.tile_pool(name="gath", bufs=4) as gp,
            tc.tile_pool(name="part", bufs=4) as pp,
            tc.tile_pool(name="mm", bufs=4) as mm,
            tc.tile_pool(name="psum", bufs=2, space="PSUM") as ps,
            tc.tile_pool(name="exp", bufs=2) as ep,
            tc.tile_pool(name="fin", bufs=2) as fp,
        ):
            # DRAM scratch
            out2_bufs = [dram.tile([BUF_ROWS, DG], F32, name=f"out2_buf{i}")
                         for i in range(2)]
            g_mine = dram.tile([R_BLK, DG], BF16)
            g_full = dram.tile([NCORES * R_BLK, DG], BF16, addr_space="Shared")
            g_table = dram.tile([NCORES * R_BLK, DG], F32)

            # ---- consts into SBUF ----
            idx_t = cp.tile([P, S2 // 16], I16)
            nc.sync.dma_start(idx_t[:], idx_in[:])
            sc_t = cp.tile([P, R2 // 16], I16)
            nc.sync.dma_start(sc_t[:], sc_in[:])
            xsl_t = cp.tile([P, S1 // P, 2], F32)
            nc.sync.dma_start(xsl_t[:], xslots_in[:])
            nm1_t = cp.tile([P, S1 // P], F32)
            nc.sync.dma_start(nm1_t[:], norm1_in[:])
            w1_f = cp.tile([P, 128], F32)
            nc.sync.dma_start(w1_f[:3, :], w1aug[:])
            w1_t = cp.tile([P, 128], BF16)
            nc.vector.tensor_copy(out=w1_t[:3, :], in_=w1_f[:3, :])
            w2_f = cp.tile([P, DG], F32)
            nc.sync.dma_start(w2_f[:], w2[:])
            w2_t = cp.tile([P, DG], BF16)
            nc.vector.tensor_copy(out=w2_t[:], in_=w2_f[:])
            wp_t = cp.tile([P, DG], F32)
            nc.sync.dma_start(wp_t[:], wp_rep[:])
            b2_t = cp.tile([P, DG], F32)
            nc.sync.dma_start(b2_t[:], b2_rep[:])
            bp_t = cp.tile([P, 1], F32)
            nc.sync.dma_start(bp_t[:], bp_rep[:])
            dv_t = cp.tile([P, G1], F32)
            nc.sync.dma_start(dv_t[:], dinv_pl[:])
            dv2p_t = cp.tile([P, G1], F32)
            nc.sync.dma_start(dv2p_t[:], dinv_pl2[:])
            xs_t = cp.tile([P, G1, 2], F32)
            nc.sync.dma_start(xs_t[:], x_self[:])
            ident = cp.tile([P, P], F32)
            make_identity(nc, ident[:])
            dv2_t = cp.tile([P, G1], F32)
            nc.vector.tensor_tensor(out=dv2_t[:], in0=dv_t[:], in1=dv_t[:],
                                    op=OP.mult)

            # ---- zero accumulator 1 (accumulator 0 is filled with g' by
            # the matmul output path) ----
            zt = cp.tile([P, 16, DG], F32)
            nc.vector.memset(zt[:], 0.0)
            bv = out2_bufs[1][:].rearrange("(g p) d -> p g d", p=P)
            g = 0
            while g < BUF_ROWS // P:
                n = min(16, BUF_ROWS // P - g)
                nc.sync.dma_start(bv[:, g:g + n, :], zt[:, :n, :])
                g += n

            # =================== phase 1: dense slot aggregation ==========
            msg_t = cp.tile([P, S1 // P, 2], F32)
            nc.vector.tensor_tensor(
                out=msg_t[:], in0=xsl_t[:],
                in1=nm1_t[:].unsqueeze(2).to_broadcast([P, S1 // P, 2]),
                op=OP.mult)
            xa_t = cp.tile([P, G1, 2], F32)
            nc.vector.memset(xa_t[:], 0.0)
            for d, gcnt, sbase, rbase in units1:
                seg = msg_t[:, sbase // P:sbase // P + gcnt * d, :]
                seg = seg.rearrange("p (g d) f -> p g f d", d=d)
                nc.vector.tensor_reduce(
                    out=xa_t[:, rbase // P:rbase // P + gcnt, :],
                    in_=seg, axis=AX.X, op=OP.add)

            # xaug = [(xa + dinv*x_self)*dinv^2, dinv]
            xaug = cp.tile([P, G1, 3], F32)
            tmp2 = mm.tile([P, G1, 2], F32, name="tmp2")
            nc.vector.tensor_tensor(
                out=tmp2[:], in0=xs_t[:],
                in1=dv_t[:].unsqueeze(2).to_broadcast([P, G1, 2]), op=OP.mult)
            nc.vector.tensor_tensor(
                out=tmp2[:], in0=tmp2[:], in1=xa_t[:], op=OP.add)
            nc.vector.tensor_tensor(
                out=xaug[:, :, 0:2], in0=tmp2[:],
                in1=dv2_t[:].unsqueeze(2).to_broadcast([P, G1, 2]), op=OP.mult)
            nc.vector.tensor_copy(out=xaug[:, :, 2:3], in_=dv_t[:].unsqueeze(2))

            # ========== mm pipeline: h1' = relu(xaug@W1aug); g' = h1'@W2 ==
            n_chunks = G1 // 4
            gm_v = g_mine[:].rearrange("(g p) d -> p g d", p=P)
            a0_v = out2_bufs[0][:R_BLK].rearrange("(g p) d -> p g d", p=P)
            for c in range(n_chunks):
                xT_ps = ps.tile([P, 512], F32, name="xTps", space="PSUM")
                for m in range(4):
                    nc.tensor.transpose(
                        out=xT_ps[0:3, m * 128:(m + 1) * 128],
                        in_=xaug[:, 4 * c + m, :], identity=ident[:])
                xT = mm.tile([P, 512], BF16, name="xT")
                nc.scalar.copy(out=xT[0:3, :], in_=xT_ps[0:3, :])
                h_ps = ps.tile([P, 512], F32, name="hps", space="PSUM")
                nc.tensor.matmul(out=h_ps[:], lhsT=w1_t[0:3, :], rhs=xT[0:3, :],
                                 start=True, stop=True)
                h1 = mm.tile([P, 512], BF16, name="h1")
                nc.scalar.activation(out=h1[:], in_=h_ps[:], func=ACTF.Relu)
                gsb = mm.tile([P, 4, DG], F32, name="gsb")
                gsb_bf = mm.tile([P, 4, DG], BF16, name="gsb_bf")
                for m in range(4):
                    g_ps = ps.tile([P, DG], F32, name="gps", space="PSUM")
                    nc.tensor.matmul(out=g_ps[:], lhsT=h1[:, m * 128:(m + 1) * 128],
                                     rhs=w2_t[:], start=True, stop=True)
                    nc.vector.tensor_copy(out=gsb[:, m, :], in_=g_ps[:])
                    nc.scalar.copy(out=gsb_bf[:, m, :], in_=g_ps[:])
                nc.sync.dma_start(gm_v[:, 4 * c:4 * c + 4, :], gsb_bf[:])
                nc.sync.dma_start(a0_v[:, 4 * c:4 * c + 4, :], gsb[:])

            # =================== allgather (bf16) ===================
            nc.gpsimd.collective_compute(
                "AllGather", mybir.AluOpType.bypass,
                replica_groups=[list(range(NCORES))],
                ins=[g_mine[:].opt()], outs=[g_full[:].opt()],
            )

            # expand each bf16 block to the f32 gather table; block b gates
            # only bucket-b gather calls, so blocks 1..7 expand under phase 2
            JW = 51
            for b in range(NCORES):
                w = 0
                while w < G1:
                    n = min(JW, G1 - w)
                    r0 = b * R_BLK + w * P
                    r1 = r0 + n * P
                    eb = ep.tile([P, JW, DG], BF16, name="eb")
                    nc.sync.dma_start(
                        eb[:, :n, :],
                        g_full[r0:r1].rearrange("(p j) d -> p j d", p=P))
                    ef = ep.tile([P, JW, DG], F32, name="ef")
                    nc.vector.tensor_copy(out=ef[:, :n, :], in_=eb[:, :n, :])
                    nc.sync.dma_start(
                        g_table[r0:r1].rearrange("(p j) d -> p j d", p=P),
                        ef[:, :n, :])
                    w += n

            # =================== phase 2: g' aggregation ===================
            for ci, call in enumerate(calls):
                b = call["bucket"]
                S = call["slots"]
                R = call["rows"]
                so, ro = call["slot_off"], call["row_off"]
                gt = gp.tile([P, CALL_MAX // P, DG], F32, name="gt")
                nc.gpsimd.dma_gather(
                    out_ap=gt[:, :S // P, :],
                    in_ap=g_table[b * R_BLK:(b + 1) * R_BLK],
                    idxs_ap=idx_t[:, so // 16:(so + S) // 16],
                    num_idxs=S, num_idxs_reg=S, elem_size=DG,
                    queue_num=0,
                )
                pt = pp.tile([P, CALL_MAX // P, DG], F32, name="pt")
                sro = 0
                rro = 0
                for d, gcnt in call["units"]:
                    seg = gt[:, sro:sro + gcnt * d, :]
                    seg = seg.rearrange("p (g d) f -> p g f d", d=d)
                    nc.vector.tensor_reduce(
                        out=pt[:, rro:rro + gcnt, :],
                        in_=seg, axis=AX.X, op=OP.add)
                    sro += gcnt * d
                    rro += gcnt
                nc.gpsimd.dma_scatter_add(
                    out_ap=out2_bufs[ci % 2][:],
                    in_ap=pt[:, :R // P, :],
                    idxs_ap=sc_t[:, ro // 16:(ro + R) // 16],
                    num_idxs=R, num_idxs_reg=R, elem_size=DG,
                    queue_num=1,
                )

            # =================== final head ===================
            # per-partition-contiguous readback: window w covers rows
            # [g*128, (g+n)*128); partition p holds rows g*128 + p*n + j
            GSTEP = 26
            g = 0
            while g < G1:
                n = min(GSTEP, G1 - g)
                r0, r1 = g * P, (g + n) * P
                o2 = fp.tile([P, GSTEP, DG], F32, name="o2")
                nc.sync.dma_start(
                    o2[:, :n, :],
                    out2_bufs[0][r0:r1].rearrange("(p j) d -> p j d", p=P))
                o2b = fp.tile([P, GSTEP, DG], F32, name="o2b")
                nc.sync.dma_start(
                    o2b[:, :n, :],
                    out2_bufs[1][r0:r1].rearrange("(p j) d -> p j d", p=P))
                nc.vector.tensor_tensor(out=o2[:, :n, :], in0=o2[:, :n, :],
                                        in1=o2b[:, :n, :], op=OP.add)
                nc.vector.tensor_tensor(
                    out=o2[:, :n, :], in0=o2[:, :n, :],
                    in1=dv2p_t[:, g:g + n].unsqueeze(2).to_broadcast([P, n, DG]),
                    op=OP.mult)
                nc.vector.tensor_tensor(
                    out=o2[:, :n, :], in0=o2[:, :n, :],
                    in1=b2_t[:].unsqueeze(1).to_broadcast([P, n, DG]), op=OP.add)
                h2 = fp.tile([P, GSTEP, DG], F32, name="h2")
                nc.scalar.activation(out=h2[:, :n, :], in_=o2[:, :n, :],
                                     func=ACTF.Relu)
                nc.vector.tensor_tensor(
                    out=h2[:, :n, :], in0=h2[:, :n, :],
                    in1=wp_t[:].unsqueeze(1).to_broadcast([P, n, DG]), op=OP.mult)
                yt = fp.tile([P, GSTEP], F32, name="yt")
                nc.vector.tensor_reduce(out=yt[:, :n], in_=h2[:, :n, :],
                                        axis=AX.X, op=OP.add)
                ys = fp.tile([P, GSTEP], F32, name="ys")
                nc.scalar.activation(out=ys[:, :n], in_=yt[:, :n],
                                     func=ACTF.Sigmoid, bias=bp_t[:, 0:1])
                nc.sync.dma_start(
                    y_out[r0:r1].rearrange("(p j) -> p j", p=P), ys[:, :n])
                g += n

    nc.compile()
    return nc


# ----------------------------------------------------------------- interface

_PROFILE = False      # set by test.py for profiled runs
LAST_EXEC_NS = None


def kernel(x, edge_index, W1, b1, W2, b2, Wp, bp):
    from concourse.bass_utils import run_bass_kernel_spmd

    x = np.asarray(x, np.float32)
    ei = np.asarray(edge_index)
    W1 = np.asarray(W1, np.float32)
    b1 = np.asarray(b1, np.float32)
    W2f = np.asarray(W2, np.float32)
    b2 = np.asarray(b2, np.float32)
    Wp = np.asarray(Wp, np.float32)
    bp = np.asarray(bp, np.float32)

    meta = _prep(ei)
    nc = _build_bass(meta)

    R_BLK = meta["R_BLK"]
    G1 = meta["G1"]
    dinv = meta["dinv"]
    row_of = meta["row_of"]
    nodes_by_core = meta["nodes_by_core"]

    w1aug = np.concatenate([W1, b1[None, :]], axis=0)
    wp_rep = np.tile(Wp[:, 0][None, :], (P, 1)).astype(np.float32)
    b2_rep = np.tile(b2[None, :], (P, 1)).astype(np.float32)
    bp_rep = np.full((P, 1), bp[0], np.float32)

    in_maps = []
    for c in range(NCORES):
        nb = nodes_by_core[c]
        rows = row_of[nb]
        dv_blk = np.zeros(R_BLK, np.float32)
        dv_blk[rows] = dinv[nb]
        dinv_pl = dv_blk.reshape(G1, P).T.copy()
        # final-head layout: window [g, g+n) -> dinv_pl2[p, g+j] = dv_blk[g*128+p*n+j]
        dinv_pl2 = np.zeros((P, G1), np.float32)
        GSTEP = 26
        g = 0
        while g < G1:
            n = min(GSTEP, G1 - g)
            dinv_pl2[:, g:g + n] = dv_blk[g * P:(g + n) * P].reshape(P, n)
            g += n
        xs = np.zeros((R_BLK, 2), np.float32)
        xs[rows] = x[nb]
        x_self_h = xs.reshape(G1, P, 2).transpose(1, 0, 2).copy()
        xsl, nm1 = meta["fill_phase1"](c, x)
        pc = meta["percore"][c]
        in_maps.append({
            "xslots": xsl, "norm1": nm1,
            "x_self": x_self_h, "dinv_pl": dinv_pl, "dinv_pl2": dinv_pl2,
            "w1aug": w1aug, "w2": W2f, "wp_rep": wp_rep,
            "b2_rep": b2_rep, "bp_rep": bp_rep,
            "idx": _wrap16(pc["idx"]), "sc": _wrap16(pc["sc"]),
        })

    global LAST_EXEC_NS
    r = run_bass_kernel_spmd(nc, in_maps, list(range(NCORES)),
                             trace=bool(_PROFILE))
    LAST_EXEC_NS = r.exec_time_ns
    y = np.zeros(N, np.float32)
    for c in range(NCORES):
        yb = r.results[c]["y"].reshape(R_BLK)
        nb = nodes_by_core[c]
        y[nb] = yb[row_of[nb]]
    return y


# revision 24
# speedup vs baseline: 1.0944x; 1.0944x over previous
"""Trainium2 Bass kernel for nn_BusStopPredictor (2-layer GCN + sigmoid head).

kernel(**inputs) takes FULL inputs, shards across 8 NeuronCores internally,
and returns the FULL [200000] output.

Strategy (graph/data parallel, dst-sharded, grid-ordered storage):
  - nodes assigned round-robin by in-degree rank (balances degree-class
    counts across cores); per-core node storage order = phase-1 grid order
    (nodes grouped by in-degree class, padded to 128-chunks)
  - phase 1 (2-wide x aggregation) uses NO indexed DMA: the per-edge
    x[src] values are materialized host-side in slot order (a static
    permutation of the input = halo materialization), so on device it is a
    dense load + DVE multiply by dinv[src] + fixed-stride segmented
    tensor_reduce producing xa directly in storage order
  - GCN algebra folded as in:
      xa    = sum dinv[s]*x[s]  (in-edges)
      xaug  = [dinv^2*(xa + dinv*x_self), dinv];  W1aug=[W1;b1]
      h1'   = relu(xaug@W1aug)  ( = dinv*h1 )
      g'    = h1'@W2            ( = dinv*(h1@W2) )  -> AllGather
      out2  = sum_in g'[s] + g'[self];  h2 = relu(dinv*out2 + b2)
      y     = sigmoid(h2@Wp + bp)
  - phase 2 (64-wide g' aggregation) via SWDGE dma_gather per src bucket
    (int16 idx, 256B rows) + degree-class tensor_reduce + dma_scatter_add
    (CCE add) into 2 ping-pong HBM accumulators; accumulator 0 is
    pre-loaded with the self term g' by the matmul output path
  - gathers on SWDGE queue 0, scatters on queue 1, with an enlarged
    descriptor ring so desc-gen is not throttled by transfer drain
"""

import numpy as np

N = 200000
NCORES = 8
NLOC = N // NCORES          # 25000
P = 128
DG = 64                     # g feature width
CALL_MAX = 1024             # max slots per SWDGE call (64 desc/lane packet cap)
SCRATCH = 49152             # SWDGE descriptor ring carveout (bytes/partition)


# ----------------------------------------------------------------- host prep

def _prep(edge_index):
    src = np.asarray(edge_index[0], np.int64)
    dst = np.asarray(edge_index[1], np.int64)
    indeg = np.bincount(dst, minlength=N)
    deg = indeg + 1
    dinv = (1.0 / np.sqrt(deg)).astype(np.float32)

    # --- node -> (core, row) assignment: round-robin by in-degree rank ---
    order = np.argsort(indeg, kind="stable")      # ascending indeg
    core_of = np.empty(N, np.int32)
    core_of[order] = np.arange(N, dtype=np.int32) % NCORES
    # per-core nodes in ascending-indeg order
    nodes_by_core = [order[core_of[order] == c] for c in range(NCORES)]

    # --- phase-1 grid: classes by total indeg, uniform chunk counts ---
    dmax1 = int(indeg.max())
    cnt1 = np.zeros((NCORES, dmax1 + 1), np.int64)
    for c in range(NCORES):
        cnt1[c] = np.bincount(indeg[nodes_by_core[c]], minlength=dmax1 + 1)
    chunks1 = np.zeros(dmax1 + 1, np.int64)
    for d in range(dmax1 + 1):
        m = int(cnt1[:, d].max())
        if m:
            chunks1[d] = (m + P - 1) // P

    grid_rows = int(chunks1.sum()) * P
    R_BLK = ((grid_rows + 511) // 512) * 512
    G1 = R_BLK // P
    TRASH = R_BLK

    # row assignment per core (all cores share class bases)
    class_base = np.zeros(dmax1 + 2, np.int64)
    np.cumsum(chunks1 * P, out=class_base[1:])
    row_of = np.full(N, -1, np.int64)     # local row within core block
    for c in range(NCORES):
        nb = nodes_by_core[c]
        degs = indeg[nb]                  # ascending
        pos = np.zeros(len(nb), np.int64)
        for d in range(dmax1 + 1):
            sel = degs == d
            k = int(sel.sum())
            if k:
                pos[sel] = class_base[d] + np.arange(k)
        row_of[nb] = pos

    # phase-1 slot layout: class-major, unit (d) has chunks1[d] chunks
    slot_base = {}
    so = 0
    units1 = []
    for d in range(1, dmax1 + 1):
        if chunks1[d]:
            slot_base[d] = so
            units1.append((d, int(chunks1[d]), so, int(class_base[d])))
            so += int(chunks1[d]) * d * P
    S1 = so
    assert S1 % P == 0

    # per-core x_slots / norm1 fill (vectorized per class)
    adj_dst_order = np.argsort(dst, kind="stable")
    es_by_dst = src[adj_dst_order]                  # srcs grouped by dst
    starts = np.zeros(N + 1, np.int64)
    np.cumsum(indeg, out=starts[1:])

    x_slots_all = []
    norm1_all = []

    def fill_phase1(c, x):
        xs = np.zeros((S1 // P, P, 2), np.float32)
        nm = np.zeros((S1 // P, P), np.float32)
        nb = nodes_by_core[c]
        degs = indeg[nb]
        for d, gcnt, sbase, rbase in units1:
            nodes = nb[degs == d]
            if len(nodes) == 0:
                continue
            # node i (i-th of this class) at chunk g=i//P, partition p=i%P
            # slots: columns sbase//P + g*d + k, partition p
            idxs = np.arange(len(nodes))
            g = idxs // P
            p = idxs % P
            # all srcs for these nodes: [len(nodes), d]
            s0 = starts[nodes]
            srcs = es_by_dst[(s0[:, None] + np.arange(d)[None, :])]
            col = sbase // P + g * d
            for k in range(d):
                xs[col + k, p] = x[srcs[:, k]]
                nm[col + k, p] = dinv[srcs[:, k]]
        return xs.transpose(1, 0, 2).copy(), nm.T.copy()

    # --- phase-2 grids: per src-bucket degree-class structure ---
    src_core = core_of[src]
    src_row = row_of[src]
    dst_core = core_of[dst]
    dst_row = row_of[dst]

    # counts[c][b] over local rows; adjacency per (c,b) sorted by dst row
    counts = np.zeros((NCORES, NCORES, R_BLK), np.int32)
    adj2 = [[None] * NCORES for _ in range(NCORES)]
    for c in range(NCORES):
        em = dst_core == c
        ed = dst_row[em]
        es = src_row[em]
        eb = src_core[em]
        for b in range(NCORES):
            bm = eb == b
            edb = ed[bm]
            esb = es[bm]
            counts[c, b] = np.bincount(edb, minlength=R_BLK)
            o = np.argsort(edb, kind="stable")
            st2 = np.zeros(R_BLK + 1, np.int64)
            np.cumsum(counts[c, b], out=st2[1:])
            adj2[c][b] = (st2, esb[o])

    calls = []
    idx_parts = [[] for _ in range(NCORES)]
    sc_parts = [[] for _ in range(NCORES)]
    slot_off = 0
    row_off = 0

    for b in range(NCORES):
        dmax = int(counts[:, b].max())
        nch = {}
        for d in range(1, dmax + 1):
            m = int(max((counts[c, b] == d).sum() for c in range(NCORES)))
            if m:
                nch[d] = (m + P - 1) // P
        nodes_by_class = []
        for c in range(NCORES):
            dloc = counts[c, b]
            nodes_by_class.append({d: np.where(dloc == d)[0] for d in nch})

        pend_units = []
        pend_slots = 0

        def flush():
            nonlocal pend_units, pend_slots, slot_off, row_off
            if not pend_units:
                return
            units = []
            for d, g in pend_units:
                if units and units[-1][0] == d:
                    units[-1][1] += 1
                else:
                    units.append([d, 1])
            rows = sum(g for _, g in units) * P
            calls.append({
                "bucket": b,
                "slots": pend_slots,
                "rows": rows,
                "units": [(d, g) for d, g in units],
                "slot_off": slot_off,
                "row_off": row_off,
            })
            slot_off += pend_slots
            row_off += rows
            pend_units = []
            pend_slots = 0

        for d in sorted(nch):
            for g in range(nch[d]):
                if pend_slots + d * P > CALL_MAX:
                    flush()
                pend_units.append((d, g))
                pend_slots += d * P
        flush()

        for c in range(NCORES):
            st2, es_s = adj2[c][b]
            for d in sorted(nch):
                nodes = nodes_by_class[c][d]
                padded = nch[d] * P
                nodes_p = np.full(padded, -1, np.int64)
                nodes_p[:len(nodes)] = nodes
                ss = np.zeros((nch[d], d, P), np.int16)
                scr = np.full((nch[d], P), TRASH, np.int16)
                if len(nodes):
                    idxs = np.arange(len(nodes))
                    g = idxs // P
                    p = idxs % P
                    s0 = st2[nodes]
                    srcs = es_s[(s0[:, None] + np.arange(d)[None, :])]
                    for k in range(d):
                        ss[g, k, p] = srcs[:, k]
                    scr[g, p] = nodes
                idx_parts[c].append(ss.reshape(-1))
                sc_parts[c].append(scr.reshape(-1))

    S2 = slot_off
    R2 = row_off

    percore = []
    for c in range(NCORES):
        percore.append({
            "idx": np.concatenate(idx_parts[c]),
            "sc": np.concatenate(sc_parts[c]),
        })

    meta = {
        "R_BLK": R_BLK, "G1": G1, "TRASH": TRASH,
        "S1": S1, "units1": units1, "S2": S2, "R2": R2,
        "calls": calls,
        "nodes_by_core": nodes_by_core, "row_of": row_of,
        "dinv": dinv, "fill_phase1": fill_phase1,
        "percore": percore,
    }
    return meta


def _wrap16(vals_i16):
    """[S] int16 -> [128, S/16] wrap layout (i%16 partition, i//16 free,
    replicated across the 8 core groups)."""
    v = np.asarray(vals_i16, np.int16)
    assert len(v) % 128 == 0
    w = v.reshape(len(v) // 16, 16).T
    return np.tile(w, (8, 1))


# ------------------------------------------------------------- device kernel

def _build_bass(meta):
    import concourse.bass as bass
    import concourse.mybir as mybir
    import concourse.tile as tile
    from concourse import bacc
    from concourse.masks import make_identity

    F32, I16 = mybir.dt.float32, mybir.dt.int16
    BF16 = mybir.dt.bfloat16
    AX = mybir.AxisListType
    OP = mybir.AluOpType
    ACTF = mybir.ActivationFunctionType

    R_BLK = meta["R_BLK"]
    G1 = meta["G1"]
    S1 = meta["S1"]
    S2 = meta["S2"]
    R2 = meta["R2"]
    units1 = meta["units1"]
    calls = meta["calls"]
    BUF_ROWS = R_BLK + P

    nc = bacc.Bacc(trn_type="TRN2", num_devices=NCORES,
                   dynamic_dma_scratch_size=32768,
                   num_swdge_queues=2)

    # inputs
    xslots_in = nc.dram_tensor("xslots", [P, S1 // P, 2], F32, kind="ExternalInput")
    norm1_in = nc.dram_tensor("norm1", [P, S1 // P], F32, kind="ExternalInput")
    x_self = nc.dram_tensor("x_self", [P, G1, 2], F32, kind="ExternalInput")
    dinv_pl = nc.dram_tensor("dinv_pl", [P, G1], F32, kind="ExternalInput")
    dinv_pl2 = nc.dram_tensor("dinv_pl2", [P, G1], F32, kind="ExternalInput")
    w1aug = nc.dram_tensor("w1aug", [3, 128], F32, kind="ExternalInput")
    w2 = nc.dram_tensor("w2", [128, DG], F32, kind="ExternalInput")
    wp_rep = nc.dram_tensor("wp_rep", [P, DG], F32, kind="ExternalInput")
    b2_rep = nc.dram_tensor("b2_rep", [P, DG], F32, kind="ExternalInput")
    bp_rep = nc.dram_tensor("bp_rep", [P, 1], F32, kind="ExternalInput")
    idx_in = nc.dram_tensor("idx", [P, S2 // 16], I16, kind="ExternalInput")
    sc_in = nc.dram_tensor("sc", [P, R2 // 16], I16, kind="ExternalInput")
    y_out = nc.dram_tensor("y", [R_BLK], F32, kind="ExternalOutput")

    with tile.TileContext(nc) as tc:
        with (
            tc.tile_pool(name="dram", bufs=1, space="DRAM") as dram,
            tc.tile_pool(name="const", bufs=1) as cp,
            tc.tile_pool(name="gath", bufs=4) as gp,
            tc.tile_pool(name="part", bufs=4) as pp,
            tc.tile_pool(name="mm", bufs=4) as mm,
            tc.tile_pool(name="psum", bufs=2, space="PSUM") as ps,
            tc.tile_pool(name="fin", bufs=3) as fp,
        ):
            # DRAM scratch
            out2_bufs = [dram.tile([BUF_ROWS, DG], F32, name=f"out2_buf{i}")
                         for i in range(2)]
            g_mine = dram.tile([R_BLK, DG], F32)
            g_full = dram.tile([NCORES * R_BLK, DG], F32, addr_space="Shared")

            # ---- consts into SBUF ----
            idx_t = cp.tile([P, S2 // 16], I16)
            nc.sync.dma_start(idx_t[:], idx_in[:])
            sc_t = cp.tile([P, R2 // 16], I16)
            nc.sync.dma_start(sc_t[:], sc_in[:])
            xsl_t = cp.tile([P, S1 // P, 2], F32)
            nc.sync.dma_start(xsl_t[:], xslots_in[:])
            nm1_t = cp.tile([P, S1 // P], F32)
            nc.sync.dma_start(nm1_t[:], norm1_in[:])
            w1_f = cp.tile([P, 128], F32)
            nc.sync.dma_start(w1_f[:3, :], w1aug[:])
            w1_t = cp.tile([P, 128], BF16)
            nc.vector.tensor_copy(out=w1_t[:3, :], in_=w1_f[:3, :])
            w2_f = cp.tile([P, DG], F32)
            nc.sync.dma_start(w2_f[:], w2[:])
            w2_t = cp.tile([P, DG], BF16)
            nc.vector.tensor_copy(out=w2_t[:], in_=w2_f[:])
            wp_t = cp.tile([P, DG], F32)
            nc.sync.dma_start(wp_t[:], wp_rep[:])
            b2_t = cp.tile([P, DG], F32)
            nc.sync.dma_start(b2_t[:], b2_rep[:])
            bp_t = cp.tile([P, 1], F32)
            nc.sync.dma_start(bp_t[:], bp_rep[:])
            dv_t = cp.tile([P, G1], F32)
            nc.sync.dma_start(dv_t[:], dinv_pl[:])
            dv2p_t = cp.tile([P, G1], F32)
            nc.sync.dma_start(dv2p_t[:], dinv_pl2[:])
            xs_t = cp.tile([P, G1, 2], F32)
            nc.sync.dma_start(xs_t[:], x_self[:])
            ident = cp.tile([P, P], F32)
            make_identity(nc, ident[:])
            dv2_t = cp.tile([P, G1], F32)
            nc.vector.tensor_tensor(out=dv2_t[:], in0=dv_t[:], in1=dv_t[:],
                                    op=OP.mult)

            # ---- zero accumulator 1 (accumulator 0 is filled with g' by
            # the matmul output path) ----
            zt = cp.tile([P, 16, DG], F32)
            nc.vector.memset(zt[:], 0.0)
            bv = out2_bufs[1][:].rearrange("(g p) d -> p g d", p=P)
            g = 0
            while g < BUF_ROWS // P:
                n = min(16, BUF_ROWS // P - g)
                nc.sync.dma_start(bv[:, g:g + n, :], zt[:, :n, :])
                g += n

            # =================== phase 1: dense slot aggregation ==========
            msg_t = cp.tile([P, S1 // P, 2], F32)
            nc.vector.tensor_tensor(
                out=msg_t[:], in0=xsl_t[:],
                in1=nm1_t[:].unsqueeze(2).to_broadcast([P, S1 // P, 2]),
                op=OP.mult)
            xa_t = cp.tile([P, G1, 2], F32)
            nc.vector.memset(xa_t[:], 0.0)
            for d, gcnt, sbase, rbase in units1:
                seg = msg_t[:, sbase // P:sbase // P + gcnt * d, :]
                seg = seg.rearrange("p (g d) f -> p g f d", d=d)
                nc.vector.tensor_reduce(
                    out=xa_t[:, rbase // P:rbase // P + gcnt, :],
                    in_=seg, axis=AX.X, op=OP.add)

            # xaug = [(xa + dinv*x_self)*dinv^2, dinv]
            xaug = cp.tile([P, G1, 3], F32)
            tmp2 = mm.tile([P, G1, 2], F32, name="tmp2")
            nc.vector.tensor_tensor(
                out=tmp2[:], in0=xs_t[:],
                in1=dv_t[:].unsqueeze(2).to_broadcast([P, G1, 2]), op=OP.mult)
            nc.vector.tensor_tensor(
                out=tmp2[:], in0=tmp2[:], in1=xa_t[:], op=OP.add)
            nc.vector.tensor_tensor(
                out=xaug[:, :, 0:2], in0=tmp2[:],
                in1=dv2_t[:].unsqueeze(2).to_broadcast([P, G1, 2]), op=OP.mult)
            nc.vector.tensor_copy(out=xaug[:, :, 2:3], in_=dv_t[:].unsqueeze(2))

            # ========== mm pipeline: h1' = relu(xaug@W1aug); g' = h1'@W2 ==
            n_chunks = G1 // 4
            gm_v = g_mine[:].rearrange("(g p) d -> p g d", p=P)
            a0_v = out2_bufs[0][:R_BLK].rearrange("(g p) d -> p g d", p=P)
            for c in range(n_chunks):
                xT_ps = ps.tile([P, 512], F32, name="xTps", space="PSUM")
                for m in range(4):
                    nc.tensor.transpose(
                        out=xT_ps[0:3, m * 128:(m + 1) * 128],
                        in_=xaug[:, 4 * c + m, :], identity=ident[:])
                xT = mm.tile([P, 512], BF16, name="xT")
                nc.scalar.copy(out=xT[0:3, :], in_=xT_ps[0:3, :])
                h_ps = ps.tile([P, 512], F32, name="hps", space="PSUM")
                nc.tensor.matmul(out=h_ps[:], lhsT=w1_t[0:3, :], rhs=xT[0:3, :],
                                 start=True, stop=True)
                h1 = mm.tile([P, 512], BF16, name="h1")
                nc.scalar.activation(out=h1[:], in_=h_ps[:], func=ACTF.Relu)
                gsb = mm.tile([P, 4, DG], F32, name="gsb")
                for m in range(4):
                    g_ps = ps.tile([P, DG], F32, name="gps", space="PSUM")
                    nc.tensor.matmul(out=g_ps[:], lhsT=h1[:, m * 128:(m + 1) * 128],
                                     rhs=w2_t[:], start=True, stop=True)
                    nc.vector.tensor_copy(out=gsb[:, m, :], in_=g_ps[:])
                nc.sync.dma_start(gm_v[:, 4 * c:4 * c + 4, :], gsb[:])
                nc.sync.dma_start(a0_v[:, 4 * c:4 * c + 4, :], gsb[:])

            # =================== allgather (bf16) ===================
            nc.gpsimd.collective_compute(
                "AllGather", mybir.AluOpType.bypass,
                replica_groups=[list(range(NCORES))],
                ins=[g_mine[:].opt()], outs=[g_full[:].opt()],
            )

            # =================== phase 2: g' aggregation ===================
            for ci, call in enumerate(calls):
                b = call["bucket"]
                S = call["slots"]
                R = call["rows"]
                so, ro = call["slot_off"], call["row_off"]
                gt = gp.tile([P, CALL_MAX // P, DG], F32, name="gt")
                nc.gpsimd.dma_gather(
                    out_ap=gt[:, :S // P, :],
                    in_ap=g_full[b * R_BLK:(b + 1) * R_BLK],
                    idxs_ap=idx_t[:, so // 16:(so + S) // 16],
                    num_idxs=S, num_idxs_reg=S, elem_size=DG,
                    queue_num=0,
                )
                pt = pp.tile([P, CALL_MAX // P, DG], F32, name="pt")
                sro = 0
                rro = 0
                for d, gcnt in call["units"]:
                    seg = gt[:, sro:sro + gcnt * d, :]
                    seg = seg.rearrange("p (g d) f -> p g f d", d=d)
                    nc.vector.tensor_reduce(
                        out=pt[:, rro:rro + gcnt, :],
                        in_=seg, axis=AX.X, op=OP.add)
                    sro += gcnt * d
                    rro += gcnt
                nc.gpsimd.dma_scatter_add(
                    out_ap=out2_bufs[ci % 2][:],
                    in_ap=pt[:, :R // P, :],
                    idxs_ap=sc_t[:, ro // 16:(ro + R) // 16],
                    num_idxs=R, num_idxs_reg=R, elem_size=DG,
                    queue_num=1,
                )

            # =================== final head ===================
            # per-partition-contiguous readback: window w covers rows
            # [g*128, (g+n)*128); partition p holds rows g*128 + p*n + j
            GSTEP = 26
            g = 0
            while g < G1:
                n = min(GSTEP, G1 - g)
                r0, r1 = g * P, (g + n) * P
                o2 = fp.tile([P, GSTEP, DG], F32, name="o2")
                nc.sync.dma_start(
                    o2[:, :n, :],
                    out2_bufs[0][r0:r1].rearrange("(p j) d -> p j d", p=P))
                o2b = fp.tile([P, GSTEP, DG], F32, name="o2b")
                nc.sync.dma_start(
                    o2b[:, :n, :],
                    out2_bufs[1][r0:r1].rearrange("(p j) d -> p j d", p=P))
                nc.vector.tensor_tensor(out=o2[:, :n, :], in0=o2[:, :n, :],
                                        in1=o2b[:, :n, :], op=OP.add)
                nc.vector.tensor_tensor(
                    out=o2[:, :n, :], in0=o2[:, :n, :],
                    in1=dv2p_t[:, g:g + n].unsqueeze(2).to_broadcast([P, n, DG]),
                    op=OP.mult)
                nc.vector.tensor_tensor(
                    out=o2[:, :n, :], in0=o2[:, :n, :],
                    in1=b2_t[:].unsqueeze(1).to_broadcast([P, n, DG]), op=OP.add)
                h2 = fp.tile([P, GSTEP, DG], F32, name="h2")
                nc.scalar.activation(out=h2[:, :n, :], in_=o2[:, :n, :],
                                     func=ACTF.Relu)
                nc.vector.tensor_tensor(
                    out=h2[:, :n, :], in0=h2[:, :n, :],
                    in1=wp_t[:].unsqueeze(1).to_broadcast([P, n, DG]), op=OP.mult)
                yt = fp.tile([P, GSTEP], F32, name="yt")
                nc.vector.tensor_reduce(out=yt[:, :n], in_=h2[:, :n, :],
                                        axis=AX.X, op=OP.add)
                ys = fp.tile([P, GSTEP], F32, name="ys")
                nc.scalar.activation(out=ys[:, :n], in_=yt[:, :n],
                                     func=ACTF.Sigmoid, bias=bp_t[:, 0:1])
                nc.sync.dma_start(
                    y_out[r0:r1].rearrange("(p j) -> p j", p=P), ys[:, :n])
                g += n

    nc.compile()
    return nc


# ----------------------------------------------------------------- interface

_PROFILE = False      # set by test.py for profiled runs
LAST_EXEC_NS = None


def kernel(x, edge_index, W1, b1, W2, b2, Wp, bp):
    from concourse.bass_utils import run_bass_kernel_spmd

    x = np.asarray(x, np.float32)
    ei = np.asarray(edge_index)
    W1 = np.asarray(W1, np.float32)
    b1 = np.asarray(b1, np.float32)
    W2f = np.asarray(W2, np.float32)
    b2 = np.asarray(b2, np.float32)
    Wp = np.asarray(Wp, np.float32)
    bp = np.asarray(bp, np.float32)

    meta = _prep(ei)
    nc = _build_bass(meta)

    R_BLK = meta["R_BLK"]
    G1 = meta["G1"]
    dinv = meta["dinv"]
    row_of = meta["row_of"]
    nodes_by_core = meta["nodes_by_core"]

    w1aug = np.concatenate([W1, b1[None, :]], axis=0)
    wp_rep = np.tile(Wp[:, 0][None, :], (P, 1)).astype(np.float32)
    b2_rep = np.tile(b2[None, :], (P, 1)).astype(np.float32)
    bp_rep = np.full((P, 1), bp[0], np.float32)

    in_maps = []
    for c in range(NCORES):
        nb = nodes_by_core[c]
        rows = row_of[nb]
        dv_blk = np.zeros(R_BLK, np.float32)
        dv_blk[rows] = dinv[nb]
        dinv_pl = dv_blk.reshape(G1, P).T.copy()
        # final-head layout: window [g, g+n) -> dinv_pl2[p, g+j] = dv_blk[g*128+p*n+j]
        dinv_pl2 = np.zeros((P, G1), np.float32)
        GSTEP = 26
        g = 0
        while g < G1:
            n = min(GSTEP, G1 - g)
            dinv_pl2[:, g:g + n] = dv_blk[g * P:(g + n) * P].reshape(P, n)
            g += n
        xs = np.zeros((R_BLK, 2), np.float32)
        xs[rows] = x[nb]
        x_self_h = xs.reshape(G1, P, 2).transpose(1, 0, 2).copy()
        xsl, nm1 = meta["fill_phase1"](c, x)
        pc = meta["percore"][c]
        in_maps.append({
            "xslots": xsl, "norm1": nm1,
            "x_self": x_self_h, "dinv_pl": dinv_pl, "dinv_pl2": dinv_pl2,
            "w1aug": w1aug, "w2": W2f, "wp_rep": wp_rep,
            "b2_rep": b2_rep, "bp_rep": bp_rep,
            "idx": _wrap16(pc["idx"]), "sc": _wrap16(pc["sc"]),
        })

    global LAST_EXEC_NS
    r = run_bass_kernel_spmd(nc, in_maps, list(range(NCORES)),
                             trace=bool(_PROFILE))
    LAST_EXEC_NS = r.exec_time_ns
    y = np.zeros(N, np.float32)
    for c in range(NCORES):
        yb = r.results[c]["y"].reshape(R_BLK)
        nb = nodes_by_core[c]
        y[nb] = yb[row_of[nb]]
    return y
